# revision 33
# baseline (speedup 1.0000x reference)
"""Trainium2 Bass kernel for streaming dot-product attention with alpha decay.

Math restructure: with e~_s = alpha^{-s} * exp(qk_s) (and noting that both the
QK_max shift and the alpha^t decay cancel in the ratio QKV_t / Z_t), the scan
  QKV_t = a*QKV_{t-1} + e_t (x) v_t ;  Z_t = a*Z_{t-1} + e_t ;  out_t = QKV_t/Z_t
becomes a pure prefix sum:
  out_t = (QKV_0 + sum_{s<=t} e~_s (x) v_s) / (Z_0 + sum_{s<=t} e~_s)
which maps onto the TensorEngine as a triangular-ones matmul over the stream
axis; the init terms enter through K=1 broadcast matmuls against an all-ones
row.  All matmuls run in fp16 with fp32 PSUM accumulation; the
reciprocal/divide path stays fp32.  Z_0 rides along as a 65th ones-column of
v_init so no separate reduction matmuls are needed.  Each core handles 8 of
the 64 batch rows (B sharded across 8 cores).

Perf deltas vs the v1 baseline (same schedule shape otherwise):
- q / k_init / k_stream arrive pre-transposed from the host, so the kernel
  does zero PE transposes and zero PSUM->SBUF staging copies for them.
- The output is written fp16 (host casts back to fp32): halves the ~17MB/core
  HBM write traffic.  Rounding adds ~5e-4 rel error, well within tolerance.
"""

import math
from contextlib import ExitStack

import numpy as np

import concourse.bass as bass
import concourse.bacc as bacc
import concourse.tile as tile
from concourse import mybir
from concourse.bass_utils import run_bass_kernel_spmd

ALPHA = 0.99
B, N1, N2, D, T = 64, 64, 512, 64, 128
NCORES = 8
BL = B // NCORES  # batch rows per core
NCH = 8           # n-chunks per b; each chunk covers 8 n values = 512 psum cols
F32 = mybir.dt.float32
F16 = mybir.dt.float16
Exp = mybir.ActivationFunctionType.Exp


def _build():
    nc = bacc.Bacc("TRN2", target_bir_lowering=False, debug=False)

    qT_d = nc.dram_tensor("qT", [D, BL, N1], F16, kind="ExternalInput")
    kinT_d = nc.dram_tensor("kinT", [D, BL, N2], F16, kind="ExternalInput")
    vin_d = nc.dram_tensor("vin", [128, BL, 4, D + 1], F16, kind="ExternalInput")
    ksT_d = nc.dram_tensor("ksT", [D, BL, T], F16, kind="ExternalInput")
    vst_d = nc.dram_tensor("vst", [T, BL, D], F16, kind="ExternalInput")
    tri_d = nc.dram_tensor("tri", [T, T], F16, kind="ExternalInput")
    sb_d = nc.dram_tensor("sbias", [T, 1], F32, kind="ExternalInput")
    out_d = nc.dram_tensor("out", [T + 1, BL, N1, D], F16, kind="ExternalOutput")

    with tile.TileContext(nc) as tc, ExitStack() as ctx:
        consts = ctx.enter_context(tc.tile_pool(name="consts", bufs=1))
        inbuf = ctx.enter_context(tc.tile_pool(name="inbuf", bufs=1))
        small = ctx.enter_context(tc.tile_pool(name="small", bufs=4))
        rbuf = ctx.enter_context(tc.tile_pool(name="rbuf", bufs=4))
        obuf = ctx.enter_context(tc.tile_pool(name="obuf", bufs=8))
        psum = ctx.enter_context(tc.tile_pool(name="psum", bufs=1, space="PSUM"))

        tri = consts.tile([T, T], F16)
        nc.sync.dma_start(out=tri[:], in_=tri_d[:])
        sbias = consts.tile([T, 1], F32)
        nc.sync.dma_start(out=sbias[:], in_=sb_d[:])

        qT_all = inbuf.tile([D, BL, N1], F16)
        kinT_all = inbuf.tile([D, BL, N2], F16)
        vin_all = inbuf.tile([128, BL, 4, D + 1], F16)
        ksT_all = inbuf.tile([D, BL, T], F16)
        vstb_all = inbuf.tile([T, BL, D], F16)
        nc.sync.dma_start(out=qT_all[:], in_=qT_d[:])
        nc.scalar.dma_start(out=kinT_all[:], in_=kinT_d[:])
        nc.scalar.dma_start(out=vin_all[:], in_=vin_d[:])
        nc.scalar.dma_start(out=ksT_all[:], in_=ksT_d[:])
        nc.sync.dma_start(out=vstb_all[:], in_=vst_d[:])

        eb_t, q0_t, rt_t = {}, {}, {}
        # Phase A: init attention, stream exponents, denominators and out0
        # for ALL rows first.  Emission order is scheduler priority, so this
        # packs the PE/ACT work densely (HAM stays warm) and has every
        # divide input ready before the heavy phase starts.
        for b in range(BL):
            # --- init attention: QKt_exp chunks [m, n], one psum bank ---
            qk_ps = psum.tile([128, 4, N1], F32, tag="pqk", bufs=2)
            for c in range(4):
                nc.tensor.matmul(
                    qk_ps[:, c, :],
                    kinT_all[:, b, 128 * c : 128 * (c + 1)],
                    qT_all[:, b, :],
                    start=True,
                    stop=True,
                )
            qke = small.tile([128, 4, N1], F16, tag="qke")
            nc.scalar.activation(qke[:], qk_ps[:], Exp)

            # [QKV_0 | Z_0] in one bank: cols 0..63 = QKV_0[n, d], col 64 = Z_0
            p0 = psum.tile([N1, D + 1], F32, tag="ptr", bufs=2)
            for c in range(4):
                nc.tensor.matmul(
                    p0[:], qke[:, c, :], vin_all[:, b, c, :],
                    start=(c == 0), stop=(c == 3),
                )

            # --- stream: e~[s, n] = exp(qk + (s+1)*(-ln a)), fp16 ---
            # (emitted before the out0 tail so R production starts early)
            ps_s = psum.tile([T, N1], F32, tag="pqk", bufs=2)
            nc.tensor.matmul(
                ps_s[:], ksT_all[:, b, :], qT_all[:, b, :], start=True, stop=True
            )
            eb = small.tile([T, N1], F16, tag="eb", bufs=8)
            nc.scalar.activation(eb[:], ps_s[:], Exp, bias=sbias[:], scale=1.0)

            # out0 = QKV_0 / Z_0  (fp32 ratio, fp16 store)
            rz = small.tile([N1, 1], F32, tag="rz")
            nc.vector.reciprocal(rz[:], p0[:, D : D + 1])
            o0 = obuf.tile([N1, D], F16, tag="o0")
            nc.scalar.activation(
                o0[:], p0[:, 0:D], mybir.ActivationFunctionType.Copy, scale=rz[:]
            )
            nc.scalar.dma_start(out=out_d[0, b], in_=o0[:])

            # fp16 copies of QKV_0 / Z_0-column for the row-0 fold-in and
            # the K=1 den-side broadcast matmul
            qkv0_h = small.tile([N1, D], F16, tag="qkv0h", bufs=8)
            nc.scalar.copy(qkv0_h[:], p0[:, 0:D])
            zcol_h = small.tile([N1, 1], F16, tag="zcolh")
            nc.scalar.copy(zcol_h[:], p0[:, D : D + 1])
            z0f = small.tile([1, N1], F16, tag="z0f")
            nc.sync.dma_start(out=z0f[:], in_=zcol_h[:, :])

            # den[t, n] = Z_0[n] + sum_{s<=t} e~[s, n]
            pden = psum.tile([T, N1], F32, tag="pqk", bufs=2)
            nc.tensor.matmul(pden[:], tri[:], eb[:], start=True, stop=False)
            nc.tensor.matmul(pden[:], tri[0:1, :], z0f[:], start=False, stop=True)
            # stage to SBUF on ACT so the 5x-faster approx reciprocal (SBUF
            # sources only) can run on DVE
            pden_sb = small.tile([T, N1], F32, tag="pdsb")
            nc.scalar.copy(pden_sb[:], pden[:])
            r_t = small.tile([T, N1], F32, tag="r", bufs=8)
            nc.vector.reciprocal_approx_fast(r_t[:], pden_sb[:])
            eb_t[b], q0_t[b], rt_t[b] = eb, qkv0_h, r_t

        # Phase B: R tensors, fold-ins, numerator matmuls, divides, stores.
        # The DVE-built R rows get their e~ broadcast pre-materialized on the
        # (largely idle) ACT engine: with a step-1 fp16 operand the R multiply
        # runs in the DVE 2x perf mode (2.25us vs 4.46us).  Emitted at phase-B
        # priority, these never preempt phase-A's critical exps.
        ebb_t = {}
        for b in range(0, BL, 2):
            ebb = rbuf.tile([T, N1, D], F16, tag="ebb", bufs=2)
            nc.scalar.activation(
                ebb[:],
                eb_t[b][:, :, None].broadcast_to([T, N1, D]),
                mybir.ActivationFunctionType.Copy,
            )
            ebb_t[b] = ebb

        for b in range(BL):
            eb, qkv0_h, r_t = eb_t[b], q0_t[b], rt_t[b]

            # R[s, n, d] = e~[s, n] * v[s, d]   (fp16)
            R_t = rbuf.tile([T, N1, D], F16, tag="R")
            if b % 2 == 0:
                nc.vector.tensor_mul(
                    R_t[:],
                    ebb_t[b][:],
                    vstb_all[:, b, None, :].broadcast_to([T, N1, D]),
                )
            else:
                nc.gpsimd.tensor_mul(
                    R_t[:],
                    eb[:, :, None].broadcast_to([T, N1, D]),
                    vstb_all[:, b, None, :].broadcast_to([T, N1, D]),
                )
            # fold QKV_0 into row s=0 (tri row 0 reaches every t)
            nc.gpsimd.dma_start(
                out=R_t[0:1, :, :], in_=qkv0_h[:, None, :],
                accum_op=mybir.AluOpType.add,
            )

            # num chunks + divide + fp16 store (output DMAs split over 2 qs)
            for c in range(NCH):
                pnum = psum.tile([T, 8, D], F32, tag="pbig", bufs=4)
                nc.tensor.matmul(
                    pnum[:], tri[:], R_t[:, 8 * c : 8 * (c + 1), :],
                    start=True, stop=True,
                )
                o_sb = obuf.tile([T, 8, D], F16, tag="osb")
                nc.vector.tensor_mul(
                    o_sb[:],
                    pnum[:],
                    r_t[:, 8 * c : 8 * (c + 1), None].broadcast_to([T, 8, D]),
                )
                eng = nc.sync if c % 2 == 0 else nc.scalar
                eng.dma_start(
                    out=out_d[1:, b, 8 * c : 8 * (c + 1), :], in_=o_sb[:]
                )

    nc.compile()
    return nc


_CACHE = {}


def _get_nc():
    if "nc" not in _CACHE:
        _CACHE["nc"] = _build()
    return _CACHE["nc"]


def _in_maps(q, k_init, v_init, k_stream, v_stream):
    q = np.asarray(q, np.float32)
    k_init = np.asarray(k_init, np.float32)
    v_init = np.asarray(v_init, np.float32)
    k_stream = np.asarray(k_stream, np.float32)
    v_stream = np.asarray(v_stream, np.float32)

    tri = np.triu(np.ones((T, T), np.float32)).astype(np.float16)
    sbias = (np.arange(1, T + 1, dtype=np.float64) * (-math.log(ALPHA))).astype(
        np.float32
    ).reshape(T, 1)

    maps = []
    for i in range(NCORES):
        sl = slice(i * BL, (i + 1) * BL)
        qs = q[sl]            # [BL, N1, D]
        kis = k_init[sl]      # [BL, N2, D]
        vis = v_init[sl]      # [BL, N2, D]
        kss = k_stream[:, sl]  # [T, BL, D]
        vss = v_stream[:, sl]  # [T, BL, D]

        qT = np.ascontiguousarray(qs.transpose(2, 0, 1)).astype(np.float16)
        kinT = np.ascontiguousarray(kis.transpose(2, 0, 1)).astype(np.float16)
        ksT = np.ascontiguousarray(kss.transpose(2, 1, 0)).astype(np.float16)

        vin = np.empty((128, BL, 4, D + 1), np.float16)
        vin[:, :, :, 0:D] = (
            vis.reshape(BL, 4, 128, D).transpose(2, 0, 1, 3).astype(np.float16)
        )
        vin[:, :, :, D] = 1.0

        vst = np.ascontiguousarray(vss).astype(np.float16)

        maps.append(
            dict(qT=qT, kinT=kinT, vin=vin, ksT=ksT, vst=vst, tri=tri,
                 sbias=sbias)
        )
    return maps


def run(q, k_init, v_init, attn_mask, k_stream, v_stream, trace=False, **trace_kw):
    """Run on hardware; returns (output, BassKernelResults)."""
    nc = _get_nc()
    maps = _in_maps(q, k_init, v_init, k_stream, v_stream)
    res = run_bass_kernel_spmd(nc, maps, list(range(NCORES)), trace=trace, **trace_kw)
    out = np.concatenate(
        [res.results[i]["out"] for i in range(NCORES)], axis=1
    ).astype(np.float32)
    return out, res


def kernel(q, k_init, v_init, attn_mask, k_stream, v_stream):
    out, _ = run(q, k_init, v_init, attn_mask, k_stream, v_stream, trace=False)
    return out


# revision 36
# speedup vs baseline: 1.0192x; 1.0192x over previous
"""Trainium2 Bass kernel for streaming dot-product attention with alpha decay.

Math restructure: with e~_s = alpha^{-s} * exp(qk_s) (and noting that both the
QK_max shift and the alpha^t decay cancel in the ratio QKV_t / Z_t), the scan
  QKV_t = a*QKV_{t-1} + e_t (x) v_t ;  Z_t = a*Z_{t-1} + e_t ;  out_t = QKV_t/Z_t
becomes a pure prefix sum:
  out_t = (QKV_0 + sum_{s<=t} e~_s (x) v_s) / (Z_0 + sum_{s<=t} e~_s)
which maps onto the TensorEngine as a triangular-ones matmul over the stream
axis; the init terms enter through K=1 broadcast matmuls against an all-ones
row.  All matmuls run in fp16 with fp32 PSUM accumulation; the
reciprocal/divide path stays fp32.  Z_0 rides along as a 65th ones-column of
v_init so no separate reduction matmuls are needed.  Each core handles 8 of
the 64 batch rows (B sharded across 8 cores).

Perf deltas vs the v1 baseline (same schedule shape otherwise):
- q / k_init / k_stream arrive pre-transposed from the host, so the kernel
  does zero PE transposes and zero PSUM->SBUF staging copies for them.
- The output is written fp16 (host casts back to fp32): halves the ~17MB/core
  HBM write traffic.  Rounding adds ~5e-4 rel error, well within tolerance.
"""

import math
from contextlib import ExitStack

import numpy as np

import concourse.bass as bass
import concourse.bacc as bacc
import concourse.tile as tile
from concourse import mybir
from concourse.bass_utils import run_bass_kernel_spmd

ALPHA = 0.99
B, N1, N2, D, T = 64, 64, 512, 64, 128
NCORES = 8
BL = B // NCORES  # batch rows per core
NCH = 8           # n-chunks per b; each chunk covers 8 n values = 512 psum cols
F32 = mybir.dt.float32
F16 = mybir.dt.float16
Exp = mybir.ActivationFunctionType.Exp


def _build():
    nc = bacc.Bacc("TRN2", target_bir_lowering=False, debug=False)

    qT_d = nc.dram_tensor("qT", [D, BL, N1], F16, kind="ExternalInput")
    kinT_d = nc.dram_tensor("kinT", [D, BL, N2], F16, kind="ExternalInput")
    vin_d = nc.dram_tensor("vin", [128, BL, 4, D + 1], F16, kind="ExternalInput")
    ksT_d = nc.dram_tensor("ksT", [D, BL, T], F16, kind="ExternalInput")
    vst_d = nc.dram_tensor("vst", [T, BL, D], F16, kind="ExternalInput")
    tri_d = nc.dram_tensor("tri", [T, T], F16, kind="ExternalInput")
    sb_d = nc.dram_tensor("sbias", [T, 1], F32, kind="ExternalInput")
    out_d = nc.dram_tensor("out", [T + 1, BL, N1, D], F16, kind="ExternalOutput")

    with tile.TileContext(nc) as tc, ExitStack() as ctx:
        consts = ctx.enter_context(tc.tile_pool(name="consts", bufs=1))
        inbuf = ctx.enter_context(tc.tile_pool(name="inbuf", bufs=1))
        small = ctx.enter_context(tc.tile_pool(name="small", bufs=4))
        rbuf = ctx.enter_context(tc.tile_pool(name="rbuf", bufs=4))
        obuf = ctx.enter_context(tc.tile_pool(name="obuf", bufs=8))
        psum = ctx.enter_context(tc.tile_pool(name="psum", bufs=1, space="PSUM"))

        tri = consts.tile([T, T], F16)
        nc.sync.dma_start(out=tri[:], in_=tri_d[:])
        sbias = consts.tile([T, 1], F32)
        nc.sync.dma_start(out=sbias[:], in_=sb_d[:])

        qT_all = inbuf.tile([D, BL, N1], F16)
        kinT_all = inbuf.tile([D, BL, N2], F16)
        vin_all = inbuf.tile([128, BL, 4, D + 1], F16)
        ksT_all = inbuf.tile([D, BL, T], F16)
        vstb_all = inbuf.tile([T, BL, D], F16)
        nc.sync.dma_start(out=qT_all[:], in_=qT_d[:])
        nc.scalar.dma_start(out=kinT_all[:], in_=kinT_d[:])
        nc.scalar.dma_start(out=vin_all[:], in_=vin_d[:])
        nc.scalar.dma_start(out=ksT_all[:], in_=ksT_d[:])
        nc.sync.dma_start(out=vstb_all[:], in_=vst_d[:])

        eb_t, q0_t, rt_t = {}, {}, {}
        # Phase A: init attention, stream exponents, denominators and out0
        # for ALL rows first.  Emission order is scheduler priority, so this
        # packs the PE/ACT work densely (HAM stays warm) and has every
        # divide input ready before the heavy phase starts.
        for b in range(BL):
            # --- init attention: QKt_exp chunks [m, n], one psum bank ---
            qk_ps = psum.tile([128, 4, N1], F32, tag="pqk", bufs=2)
            for c in range(4):
                nc.tensor.matmul(
                    qk_ps[:, c, :],
                    kinT_all[:, b, 128 * c : 128 * (c + 1)],
                    qT_all[:, b, :],
                    start=True,
                    stop=True,
                )
            qke = small.tile([128, 4, N1], F16, tag="qke")
            nc.scalar.activation(qke[:], qk_ps[:], Exp)

            # [QKV_0 | Z_0] in one bank: cols 0..63 = QKV_0[n, d], col 64 = Z_0
            p0 = psum.tile([N1, D + 1], F32, tag="ptr", bufs=2)
            for c in range(4):
                nc.tensor.matmul(
                    p0[:], qke[:, c, :], vin_all[:, b, c, :],
                    start=(c == 0), stop=(c == 3),
                )

            # --- stream: e~[s, n] = exp(qk + (s+1)*(-ln a)), fp16 ---
            # (emitted before the out0 tail so R production starts early)
            ps_s = psum.tile([T, N1], F32, tag="pqk", bufs=2)
            nc.tensor.matmul(
                ps_s[:], ksT_all[:, b, :], qT_all[:, b, :], start=True, stop=True
            )
            eb = small.tile([T, N1], F16, tag="eb", bufs=8)
            nc.scalar.activation(eb[:], ps_s[:], Exp, bias=sbias[:], scale=1.0)

            # out0 = QKV_0 / Z_0  (fp32 ratio, fp16 store)
            rz = small.tile([N1, 1], F32, tag="rz")
            nc.vector.reciprocal(rz[:], p0[:, D : D + 1])
            o0 = obuf.tile([N1, D], F16, tag="o0")
            nc.vector.tensor_scalar_mul(o0[:], p0[:, 0:D], rz[:])
            nc.scalar.dma_start(out=out_d[0, b], in_=o0[:])

            # fp16 copies of QKV_0 / Z_0-column for the row-0 fold-in and
            # the K=1 den-side broadcast matmul
            qkv0_h = small.tile([N1, D], F16, tag="qkv0h", bufs=8)
            nc.scalar.copy(qkv0_h[:], p0[:, 0:D])
            zcol_h = small.tile([N1, 1], F16, tag="zcolh")
            nc.scalar.copy(zcol_h[:], p0[:, D : D + 1])
            z0f = small.tile([1, N1], F16, tag="z0f")
            nc.sync.dma_start(out=z0f[:], in_=zcol_h[:, :])

            # den[t, n] = Z_0[n] + sum_{s<=t} e~[s, n]
            pden = psum.tile([T, N1], F32, tag="pqk", bufs=2)
            nc.tensor.matmul(pden[:], tri[:], eb[:], start=True, stop=False)
            nc.tensor.matmul(pden[:], tri[0:1, :], z0f[:], start=False, stop=True)
            # stage to SBUF on ACT so the 5x-faster approx reciprocal (SBUF
            # sources only) can run on DVE
            pden_sb = small.tile([T, N1], F32, tag="pdsb")
            nc.scalar.copy(pden_sb[:], pden[:])
            r_t = small.tile([T, N1], F32, tag="r", bufs=8)
            nc.vector.reciprocal_approx_fast(r_t[:], pden_sb[:])
            eb_t[b], q0_t[b], rt_t[b] = eb, qkv0_h, r_t

        # Phase B: R tensors, fold-ins, numerator matmuls, divides, stores.
        # The DVE-built R rows get their e~ broadcast pre-materialized on the
        # (largely idle) ACT engine: with a step-1 fp16 operand the R multiply
        # runs in the DVE 2x perf mode (2.25us vs 4.46us).  Emitted at phase-B
        # priority, these never preempt phase-A's critical exps.
        ebb_t = {}
        for b in range(0, BL, 2):
            ebb = rbuf.tile([T, N1, D], F16, tag="ebb", bufs=2)
            nc.scalar.activation(
                ebb[:],
                eb_t[b][:, :, None].broadcast_to([T, N1, D]),
                mybir.ActivationFunctionType.Copy,
            )
            ebb_t[b] = ebb

        for b in range(BL):
            eb, qkv0_h, r_t = eb_t[b], q0_t[b], rt_t[b]

            # R[s, n, d] = e~[s, n] * v[s, d]   (fp16)
            R_t = rbuf.tile([T, N1, D], F16, tag="R")
            if b % 2 == 0:
                nc.vector.tensor_mul(
                    R_t[:],
                    ebb_t[b][:],
                    vstb_all[:, b, None, :].broadcast_to([T, N1, D]),
                )
            else:
                nc.gpsimd.tensor_mul(
                    R_t[:],
                    eb[:, :, None].broadcast_to([T, N1, D]),
                    vstb_all[:, b, None, :].broadcast_to([T, N1, D]),
                )
            # fold QKV_0 into row s=0 (tri row 0 reaches every t)
            nc.gpsimd.dma_start(
                out=R_t[0:1, :, :], in_=qkv0_h[:, None, :],
                accum_op=mybir.AluOpType.add,
            )

            # num chunks + divide + fp16 store (output DMAs split over 2 qs)
            for c in range(NCH):
                pnum = psum.tile([T, 8, D], F32, tag="pbig", bufs=4)
                nc.tensor.matmul(
                    pnum[:], tri[:], R_t[:, 8 * c : 8 * (c + 1), :],
                    start=True, stop=True,
                )
                o_sb = obuf.tile([T, 8, D], F16, tag="osb")
                nc.vector.tensor_mul(
                    o_sb[:],
                    pnum[:],
                    r_t[:, 8 * c : 8 * (c + 1), None].broadcast_to([T, 8, D]),
                )
                eng = nc.sync if c % 2 == 0 else nc.scalar
                eng.dma_start(
                    out=out_d[1:, b, 8 * c : 8 * (c + 1), :], in_=o_sb[:]
                )

    nc.compile()
    return nc


_CACHE = {}


def _get_nc():
    if "nc" not in _CACHE:
        _CACHE["nc"] = _build()
    return _CACHE["nc"]


def _in_maps(q, k_init, v_init, k_stream, v_stream):
    q = np.asarray(q, np.float32)
    k_init = np.asarray(k_init, np.float32)
    v_init = np.asarray(v_init, np.float32)
    k_stream = np.asarray(k_stream, np.float32)
    v_stream = np.asarray(v_stream, np.float32)

    tri = np.triu(np.ones((T, T), np.float32)).astype(np.float16)
    sbias = (np.arange(1, T + 1, dtype=np.float64) * (-math.log(ALPHA))).astype(
        np.float32
    ).reshape(T, 1)

    maps = []
    for i in range(NCORES):
        sl = slice(i * BL, (i + 1) * BL)
        qs = q[sl]            # [BL, N1, D]
        kis = k_init[sl]      # [BL, N2, D]
        vis = v_init[sl]      # [BL, N2, D]
        kss = k_stream[:, sl]  # [T, BL, D]
        vss = v_stream[:, sl]  # [T, BL, D]

        qT = np.ascontiguousarray(qs.transpose(2, 0, 1)).astype(np.float16)
        kinT = np.ascontiguousarray(kis.transpose(2, 0, 1)).astype(np.float16)
        ksT = np.ascontiguousarray(kss.transpose(2, 1, 0)).astype(np.float16)

        vin = np.empty((128, BL, 4, D + 1), np.float16)
        vin[:, :, :, 0:D] = (
            vis.reshape(BL, 4, 128, D).transpose(2, 0, 1, 3).astype(np.float16)
        )
        vin[:, :, :, D] = 1.0

        vst = np.ascontiguousarray(vss).astype(np.float16)

        maps.append(
            dict(qT=qT, kinT=kinT, vin=vin, ksT=ksT, vst=vst, tri=tri,
                 sbias=sbias)
        )
    return maps


def run(q, k_init, v_init, attn_mask, k_stream, v_stream, trace=False, **trace_kw):
    """Run on hardware; returns (output, BassKernelResults)."""
    nc = _get_nc()
    maps = _in_maps(q, k_init, v_init, k_stream, v_stream)
    res = run_bass_kernel_spmd(nc, maps, list(range(NCORES)), trace=trace, **trace_kw)
    out = np.concatenate(
        [res.results[i]["out"] for i in range(NCORES)], axis=1
    ).astype(np.float32)
    return out, res


def kernel(q, k_init, v_init, attn_mask, k_stream, v_stream):
    out, _ = run(q, k_init, v_init, attn_mask, k_stream, v_stream, trace=False)
    return out
